# revision 35
# baseline (speedup 1.0000x reference)
"""Multi-head causal attention (B=4, S=4096, E=512, H=8) on 8 trn2 NeuronCores.

Sharding: core = (batch b, head-group g of 4 heads); 4 batches x 2 groups = 8 cores.
Each core computes qkv projection for its group's heads, causal attention, and a
partial output projection (its heads' rows of Wo). Host sums the two partials per
batch and adds bo.

Device layout (per core):
  xT   [512, 4096]   x[b] transposed (host-side) -> contraction dim on partitions
  qT/kT stored [128(2 heads' dh), 1024-token tiles]  (qkvT = W.T @ x.T on PE)
  V    stored token-major [128, kb*260 + h*65 + d] with a ones column per
       (kb, head) at d=64 -> the PV matmul lhsT [Vh|1] yields attention output
       in [dh, tok] layout AND softmax denominators in one pass.
  S_T  [128 keys, 1024 queries] in PSUM per 128-key block, causal-trapezoid
       column ranges; exp on ACT (scale=1/8 folded in); PV accumulates over
       key blocks in PSUM.

All matmul operands are bf16 (PSUM accumulation stays fp32). The causal mask
for diagonal blocks is applied by a DVE triangle-multiply on the exp output
(cheaper than PE mask matmuls under the sustained K=4/8 PE clock throttle).
Epilogue runs entirely against the PSUM accumulator: DVE reciprocal of the
sums row, PE broadcast of the reciprocal into the tile's unused partitions
64:128, one DVE multiply out — no DMA round-trip on the critical path.
Wo partials DMA straight from PSUM to DRAM.
"""

import sys

sys.path.insert(0, "/opt/trn_rl_repo")

import numpy as np

B, S, E = 4, 4096, 512
H = 8
DH = 64
HPG = 4  # heads per group
GQ = 256  # features per group for each of q/k/v (HPG*DH)
QE = 1024  # query extent per attention sweep
NQQ = S // QE  # 4
NKB = S // 128  # 32
NTQ = 4  # token chunks for projection phase
TQ = S // NTQ  # 1024
VW = HPG * 65  # 260: per-key-block V width incl. ones columns
SCALE = 0.125  # 1/sqrt(DH)

_CACHE = {}


def _chunks(qs, hi):
    """Split [qs, hi) into pieces that never cross a 512-column PSUM bank
    boundary (one matmul output must stay within a single PSUM bank)."""
    out = []
    for c0 in range(0, hi, 512):
        j0, j1 = max(qs, c0), min(hi, c0 + 512)
        if j0 < j1:
            out.append((j0, j1))
    return out



def _build_nc(repeat=1):
    import concourse.bass as bass
    import concourse.tile as tile
    import concourse.mybir as mybir
    from concourse import bacc

    f32 = mybir.dt.float32
    f32r = mybir.dt.float32r
    bf16 = mybir.dt.bfloat16
    AF = mybir.ActivationFunctionType
    ALU = mybir.AluOpType

    nc = bacc.Bacc("TRN2", target_bir_lowering=False, debug=False)

    xT = nc.dram_tensor("xT", [E, S], bf16, kind="ExternalInput").ap()
    wqk = nc.dram_tensor("wqk", [E, 512], bf16, kind="ExternalInput").ap()
    bqk = nc.dram_tensor("bqk", [128, 4], f32, kind="ExternalInput").ap()
    wv = nc.dram_tensor("wv", [E, GQ], bf16, kind="ExternalInput").ap()
    bv = nc.dram_tensor("bv", [1, GQ], bf16, kind="ExternalInput").ap()
    wo = nc.dram_tensor("wo", [DH, HPG * 512], bf16, kind="ExternalInput").ap()
    out = nc.dram_tensor("out", [S, E], f32, kind="ExternalOutput").ap()

    with tile.TileContext(nc) as tc:
        with (
            tc.tile_pool(name="consts", bufs=1) as cpool,
            tc.tile_pool(name="xt", bufs=8) as xtpool,
            tc.tile_pool(name="qkv", bufs=1) as qkvpool,
            tc.tile_pool(name="pt", bufs=4) as ptpool,
            tc.tile_pool(name="att", bufs=1) as attpool,
            tc.tile_pool(name="eps", bufs=3) as epool,
            tc.tile_pool(name="outs", bufs=1) as opool,
            # PSUM: 8 banks fully owned by the paired attention loops;
            # projection/Wo psum tiles share the same slots via tags.
            tc.tile_pool(name="st", bufs=1, space="PSUM") as stpool,
            tc.tile_pool(name="ov", bufs=1, space="PSUM") as ovpool,
        ):
            # ---- constants ----
            wqk_sb = cpool.tile([128, 4 * 512], bf16, name="wqk_sb")
            for ec in range(4):
                nc.sync.dma_start(
                    wqk_sb[:, ec * 512 : (ec + 1) * 512],
                    wqk[ec * 128 : (ec + 1) * 128, :],
                )
            wv_sb = cpool.tile([128, 4 * GQ], bf16, name="wv_sb")
            for ec in range(4):
                nc.sync.dma_start(
                    wv_sb[:, ec * GQ : (ec + 1) * GQ],
                    wv[ec * 128 : (ec + 1) * 128, :],
                )
            wo_sb = cpool.tile([DH, HPG * 512], bf16, name="wo_sb")
            nc.sync.dma_start(wo_sb[:], wo[:])
            bqk_sb = cpool.tile([128, 4], f32, name="bqk_sb")
            nc.sync.dma_start(bqk_sb[:], bqk[:])
            bv_sb = cpool.tile([1, GQ], bf16, name="bv_sb")
            nc.sync.dma_start(bv_sb[:], bv[:])
            bv_bc = cpool.tile([128, GQ], bf16, name="bv_bc")
            nc.sync.dma_start(
                bv_bc[:], bv_sb[0:1, :].unsqueeze(1).to_broadcast([1, 128, GQ])
            )
            onesf = cpool.tile([128, 128], f32, name="onesf")
            nc.vector.memset(onesf[:], 1.0)
            ones64b = cpool.tile([1, DH], bf16, name="ones64b")
            nc.vector.memset(ones64b[:], 1.0)
            # tri01[p, j] = 1.0 where j >= p (causal-allowed), else 0
            trif = cpool.tile([128, 128], f32, name="trif")
            nc.vector.memset(trif[:], 1.0)
            nc.gpsimd.affine_select(
                out=trif[:], in_=trif[:], compare_op=ALU.is_ge, fill=0.0,
                base=0, pattern=[[1, 128]], channel_multiplier=-1,
            )
            tri01 = cpool.tile([128, 128], bf16, name="tri01")
            nc.vector.tensor_copy(tri01[:], trif[:])

            # persistent qT/kT tiles: [pair A/B][tq] each [128, 1024]
            # pair A rows 0:64 = head0 dh, 64:128 = head1; pair B = heads 2,3
            qt = [
                [qkvpool.tile([128, TQ], bf16, name=f"qt{ab}_{t}") for t in range(NTQ)]
                for ab in range(2)
            ]
            kt = [
                [qkvpool.tile([128, TQ], bf16, name=f"kt{ab}_{t}") for t in range(NTQ)]
                for ab in range(2)
            ]
            vt = [
                qkvpool.tile([128, 8 * VW], bf16, name=f"vt_{t}") for t in range(NTQ)
            ]

            def p1(tq):
                xts = []
                for ec in range(4):
                    xtile = xtpool.tile([128, TQ], bf16, name="xtile", tag="xtile")
                    nc.sync.dma_start(
                        xtile[:],
                        xT[ec * 128 : (ec + 1) * 128, tq * TQ : (tq + 1) * TQ],
                    )
                    xts.append(xtile)
                for gi, fc in enumerate((0, 2, 1, 3)):
                    dest = (qt if fc < 2 else kt)[fc % 2][tq]
                    for th in range(2):
                        tag = ("st_e", "st_o")[(gi * 2 + th) % 2]
                        ps = stpool.tile([128, 512], f32, name="mmps", tag=tag)
                        for ec in range(4):
                            nc.tensor.matmul(
                                ps[:],
                                lhsT=wqk_sb[:, ec * 512 + fc * 128 : ec * 512 + (fc + 1) * 128],
                                rhs=xts[ec][:, th * 512 : (th + 1) * 512],
                                start=(ec == 0),
                                stop=(ec == 3),
                            )
                        nc.vector.tensor_scalar_add(
                            dest[:, th * 512 : (th + 1) * 512],
                            ps[:],
                            bqk_sb[:, fc : fc + 1],
                        )
                v_tile = vt[tq]
                nc.vector.tensor_copy(
                    v_tile.rearrange("p (t h d) -> p t h d", t=8, h=HPG)[:, :, :, 64:65],
                    onesf[:, 0:32].rearrange("p (t h d) -> p t h d", t=8, h=HPG),
                )
                for tb in range(8):
                    vps = ovpool.tile(
                        [128, GQ], f32, name="vps", tag=("ov_e", "ov_o")[tb % 2]
                    )
                    for ec in range(4):
                        nc.tensor.matmul(
                            vps[:],
                            lhsT=xts[ec][:, tb * 128 : (tb + 1) * 128],
                            rhs=wv_sb[:, ec * GQ : (ec + 1) * GQ],
                            start=(ec == 0),
                            stop=(ec == 3),
                        )
                    nc.vector.tensor_tensor(
                        v_tile[:, tb * VW : (tb + 1) * VW].rearrange(
                            "p (h d) -> p h d", h=HPG
                        )[:, :, 0:64],
                        vps.rearrange("p (h d) -> p h d", h=HPG),
                        bv_bc.rearrange("p (h d) -> p h d", h=HPG),
                        ALU.add,
                    )

            atts = {}

            def epilogue(att_tile, ovt):
                # DVE copies + short reciprocal on the sums row; the reciprocal
                # is broadcast across partitions by a sync-engine DMA (the PE
                # never touches the epilogue), then one DVE mult frees ov.
                occ = epool.tile([DH, QE], f32, name="occ", tag="occ")
                nc.vector.tensor_copy(occ[:], ovt[0:64, :])
                # custom-DVE ISA ops (reciprocal_approx_*) drop the partition
                # offset of their input AP — stage the sums row at base 0.
                srow = epool.tile([1, QE], f32, name="srow", tag="srow")
                nc.vector.tensor_copy(srow[:], ovt[64:65, :])
                rbc = epool.tile([1, QE], f32, name="rbc", tag="rbc")
                nc.vector.reciprocal_approx_fast(out=rbc[:], in_=srow[:])
                sbc = epool.tile([DH, QE], f32, name="sbc", tag="sbc")
                nc.sync.dma_start(
                    sbc[:], rbc[0:1, :].unsqueeze(1).to_broadcast([1, DH, QE])
                )
                nc.vector.tensor_tensor(att_tile[:], occ[:], sbc[:], ALU.mult)

            def att(qq, mid=None):
                atts[qq] = [
                    attpool.tile([DH, QE], bf16, name=f"att_h{h}", tag=f"att{h}")
                    for h in range(HPG)
                ]
                nkb = 8 * qq + 8
                for pr in range(2):  # head pair (2pr, 2pr+1)
                    if pr == 1 and mid is not None:
                        mid()
                    ov_e = ovpool.tile([128, QE], f32, name="ov_e", tag="ov_e")
                    ov_o = ovpool.tile([128, QE], f32, name="ov_o", tag="ov_o")
                    for kb in range(nkb):
                        tqk, kbl = kb // 8, kb % 8
                        qs = max(0, (kb - 8 * qq) * 128)
                        st_e = stpool.tile([128, QE], f32, name="st_e", tag="st_e")
                        st_o = stpool.tile([128, QE], f32, name="st_o", tag="st_o")
                        for j0, j1 in _chunks(qs, QE):
                            # two row-tiled matmuls (rows 0:64 / 64:128)
                            nc.tensor.matmul(
                                st_e[:, j0:j1],
                                lhsT=kt[pr][tqk][0:64, kbl * 128 : (kbl + 1) * 128],
                                rhs=qt[pr][qq][0:64, j0:j1],
                                start=True,
                                stop=True,
                            )
                            nc.tensor.matmul(
                                st_o[:, j0:j1],
                                lhsT=kt[pr][tqk][64:128, kbl * 128 : (kbl + 1) * 128],
                                rhs=qt[pr][qq][64:128, j0:j1],
                                start=True,
                                stop=True,
                            )
                        pt_e = ptpool.tile([128, QE], bf16, name="pt_e", tag="pt")
                        pt_o = ptpool.tile([128, QE], bf16, name="pt_o", tag="pt")
                        nc.scalar.activation(
                            pt_e[:, qs:QE], st_e[:, qs:QE], AF.Exp, bias=0.0, scale=SCALE
                        )
                        nc.scalar.activation(
                            pt_o[:, qs:QE], st_o[:, qs:QE], AF.Exp, bias=0.0, scale=SCALE
                        )
                        if kb >= 8 * qq:  # diagonal: zero the causally-forbidden
                            for ptx in (pt_e, pt_o):  # upper triangle on DVE
                                nc.vector.tensor_tensor(
                                    ptx[:, qs : qs + 128],
                                    ptx[:, qs : qs + 128],
                                    tri01[:],
                                    ALU.mult,
                                )
                        for j0, j1 in _chunks(qs, QE):
                            nc.tensor.matmul(
                                ov_e[0:65, j0:j1],
                                lhsT=vt[tqk][:, kbl * VW + 2 * pr * 65 : kbl * VW + (2 * pr + 1) * 65],
                                rhs=pt_e[:, j0:j1],
                                start=(kb == 0),
                                stop=(kb == nkb - 1),
                                skip_group_check=True,
                            )
                            nc.tensor.matmul(
                                ov_o[0:65, j0:j1],
                                lhsT=vt[tqk][:, kbl * VW + (2 * pr + 1) * 65 : kbl * VW + (2 * pr + 2) * 65],
                                rhs=pt_o[:, j0:j1],
                                start=(kb == 0),
                                stop=(kb == nkb - 1),
                                skip_group_check=True,
                            )
                    epilogue(atts[qq][2 * pr], ov_e)
                    epilogue(atts[qq][2 * pr + 1], ov_o)

            def wo(qq):
                att_h = atts[qq]
                out_sb = opool.tile([128, 4 * 512], f32, name="out_sb", tag="osb")
                for half in range(2):
                    for tb4 in range(4):
                        tb = half * 4 + tb4
                        wops = stpool.tile(
                            [128, 512], f32, name="wops", tag=("st_e", "st_o")[tb4 % 2]
                        )
                        for h in range(HPG):
                            nc.tensor.matmul(
                                wops[:],
                                lhsT=att_h[h][:, tb * 128 : (tb + 1) * 128],
                                rhs=wo_sb[:, h * 512 : (h + 1) * 512],
                                start=(h == 0),
                                stop=(h == HPG - 1),
                            )
                        # evacuate PSUM on the scalar engine (ACT is idle here;
                        # keeps the in-order DVE queue free for epilogues)
                        nc.scalar.copy(
                            out_sb[:, tb4 * 512 : (tb4 + 1) * 512], wops[:]
                        )
                    nc.sync.dma_start(
                        out[
                            qq * QE + half * 512 : qq * QE + (half + 1) * 512, :
                        ].rearrange("(t p) c -> p t c", p=128),
                        out_sb.rearrange("p (t c) -> p t c", t=4),
                    )

            def body(_i=None):
                for tq in range(NTQ):
                    p1(tq)
                att(0)
                att(1, mid=lambda: wo(0))
                att(2, mid=lambda: wo(1))
                att(3, mid=lambda: wo(2))
                wo(3)

            if repeat == 1:
                body()
            else:
                with tc.For_i(0, repeat, 1) as _i:
                    body(_i)

    nc.finalize()
    return nc


def _get_nc(repeat=1):
    key = ("nc", repeat)
    if key not in _CACHE:
        _CACHE[key] = _build_nc(repeat)
    return _CACHE[key]


def _make_in_maps(x, Wqkv, bqkv, Wo):
    from ml_dtypes import bfloat16

    in_maps = []
    for core in range(8):
        b, g = core // 2, core % 2
        qs, ks, vs = g * GQ, 512 + g * GQ, 1024 + g * GQ
        wqk_np = np.ascontiguousarray(
            np.concatenate([Wqkv[:, qs : qs + GQ], Wqkv[:, ks : ks + GQ]], axis=1)
        ).astype(bfloat16)
        bqk_np = np.ascontiguousarray(
            np.concatenate([bqkv[qs : qs + GQ], bqkv[ks : ks + GQ]]).reshape(4, 128).T
        )
        wv_np = np.ascontiguousarray(Wqkv[:, vs : vs + GQ]).astype(bfloat16)
        bv_np = np.ascontiguousarray(bqkv[vs : vs + GQ].reshape(1, GQ)).astype(bfloat16)
        wo_g = Wo[g * GQ : (g + 1) * GQ, :]
        wo_np = np.ascontiguousarray(
            np.concatenate([wo_g[h * DH : (h + 1) * DH, :] for h in range(HPG)], axis=1)
        ).astype(bfloat16)
        in_maps.append(
            {
                "xT": np.ascontiguousarray(x[b].T).astype(bfloat16),
                "wqk": wqk_np,
                "bqk": bqk_np,
                "wv": wv_np,
                "bv": bv_np,
                "wo": wo_np,
            }
        )
    return in_maps


def kernel(x, Wqkv, bqkv, Wo, bo, **run_kwargs):
    from concourse.bass_utils import run_bass_kernel_spmd

    x = np.asarray(x, dtype=np.float32)
    Wqkv = np.asarray(Wqkv, dtype=np.float32)
    bqkv = np.asarray(bqkv, dtype=np.float32)
    Wo = np.asarray(Wo, dtype=np.float32)
    bo = np.asarray(bo, dtype=np.float32)

    nc = _get_nc()
    in_maps = _make_in_maps(x, Wqkv, bqkv, Wo)

    res = run_bass_kernel_spmd(nc, in_maps, core_ids=list(range(8)), **run_kwargs)
    _CACHE["last_results"] = res

    out = np.empty((B, S, E), dtype=np.float32)
    for b in range(B):
        out[b] = res.results[2 * b]["out"] + res.results[2 * b + 1]["out"] + bo
    return out


# revision 36
# speedup vs baseline: 1.0632x; 1.0632x over previous
"""Multi-head causal attention (B=4, S=4096, E=512, H=8) on 8 trn2 NeuronCores.

Sharding: core = (batch b, head-group g of 4 heads); 4 batches x 2 groups = 8 cores.
Each core computes qkv projection for its group's heads, causal attention, and a
partial output projection (its heads' rows of Wo). Host sums the two partials per
batch and adds bo.

Device layout (per core):
  xT   [512, 4096]   x[b] transposed (host-side) -> contraction dim on partitions
  qT/kT stored [128(2 heads' dh), 1024-token tiles]  (qkvT = W.T @ x.T on PE)
  V    stored token-major [128, kb*260 + h*65 + d] with a ones column per
       (kb, head) at d=64 -> the PV matmul lhsT [Vh|1] yields attention output
       in [dh, tok] layout AND softmax denominators in one pass.
  S_T  [128 keys, 1024 queries] in PSUM per 128-key block, causal-trapezoid
       column ranges; exp on ACT (scale=1/8 folded in); PV accumulates over
       key blocks in PSUM.

All matmul operands are bf16 (PSUM accumulation stays fp32). The causal mask
for diagonal blocks is applied by a DVE triangle-multiply on the exp output
(cheaper than PE mask matmuls under the sustained K=4/8 PE clock throttle).
Epilogue runs entirely against the PSUM accumulator: DVE reciprocal of the
sums row, PE broadcast of the reciprocal into the tile's unused partitions
64:128, one DVE multiply out — no DMA round-trip on the critical path.
Wo partials DMA straight from PSUM to DRAM.
"""

import sys

sys.path.insert(0, "/opt/trn_rl_repo")

import numpy as np

B, S, E = 4, 4096, 512
H = 8
DH = 64
HPG = 4  # heads per group
GQ = 256  # features per group for each of q/k/v (HPG*DH)
QE = 1024  # query extent per attention sweep
NQQ = S // QE  # 4
NKB = S // 128  # 32
NTQ = 4  # token chunks for projection phase
TQ = S // NTQ  # 1024
VW = HPG * 65  # 260: per-key-block V width incl. ones columns
SCALE = 0.125  # 1/sqrt(DH)

_CACHE = {}


def _chunks(qs, hi):
    """Split [qs, hi) into pieces that never cross a 512-column PSUM bank
    boundary (one matmul output must stay within a single PSUM bank)."""
    out = []
    for c0 in range(0, hi, 512):
        j0, j1 = max(qs, c0), min(hi, c0 + 512)
        if j0 < j1:
            out.append((j0, j1))
    return out



def _build_nc(repeat=1):
    import concourse.bass as bass
    import concourse.tile as tile
    import concourse.mybir as mybir
    from concourse import bacc

    f32 = mybir.dt.float32
    f32r = mybir.dt.float32r
    bf16 = mybir.dt.bfloat16
    AF = mybir.ActivationFunctionType
    ALU = mybir.AluOpType

    nc = bacc.Bacc("TRN2", target_bir_lowering=False, debug=False)

    xT = nc.dram_tensor("xT", [E, S], bf16, kind="ExternalInput").ap()
    wqk = nc.dram_tensor("wqk", [E, 512], bf16, kind="ExternalInput").ap()
    bqk = nc.dram_tensor("bqk", [128, 4], f32, kind="ExternalInput").ap()
    wv = nc.dram_tensor("wv", [E, GQ], bf16, kind="ExternalInput").ap()
    bv = nc.dram_tensor("bv", [1, GQ], bf16, kind="ExternalInput").ap()
    wo = nc.dram_tensor("wo", [DH, HPG * 512], bf16, kind="ExternalInput").ap()
    out = nc.dram_tensor("out", [S, E], f32, kind="ExternalOutput").ap()

    with tile.TileContext(nc) as tc:
        with (
            tc.tile_pool(name="consts", bufs=1) as cpool,
            tc.tile_pool(name="xt", bufs=8) as xtpool,
            tc.tile_pool(name="qkv", bufs=1) as qkvpool,
            tc.tile_pool(name="pt", bufs=4) as ptpool,
            tc.tile_pool(name="att", bufs=1) as attpool,
            tc.tile_pool(name="eps", bufs=3) as epool,
            tc.tile_pool(name="outs", bufs=1) as opool,
            # PSUM: 8 banks fully owned by the paired attention loops;
            # projection/Wo psum tiles share the same slots via tags.
            tc.tile_pool(name="st", bufs=1, space="PSUM") as stpool,
            tc.tile_pool(name="ov", bufs=1, space="PSUM") as ovpool,
        ):
            # ---- constants ----
            wqk_sb = cpool.tile([128, 4 * 512], bf16, name="wqk_sb")
            for ec in range(4):
                nc.sync.dma_start(
                    wqk_sb[:, ec * 512 : (ec + 1) * 512],
                    wqk[ec * 128 : (ec + 1) * 128, :],
                )
            wv_sb = cpool.tile([128, 4 * GQ], bf16, name="wv_sb")
            for ec in range(4):
                nc.sync.dma_start(
                    wv_sb[:, ec * GQ : (ec + 1) * GQ],
                    wv[ec * 128 : (ec + 1) * 128, :],
                )
            wo_sb = cpool.tile([DH, HPG * 512], bf16, name="wo_sb")
            nc.sync.dma_start(wo_sb[:], wo[:])
            bqk_sb = cpool.tile([128, 4], f32, name="bqk_sb")
            nc.sync.dma_start(bqk_sb[:], bqk[:])
            bv_sb = cpool.tile([1, GQ], bf16, name="bv_sb")
            nc.sync.dma_start(bv_sb[:], bv[:])
            bv_bc = cpool.tile([128, GQ], bf16, name="bv_bc")
            nc.sync.dma_start(
                bv_bc[:], bv_sb[0:1, :].unsqueeze(1).to_broadcast([1, 128, GQ])
            )
            onesf = cpool.tile([128, 128], f32, name="onesf")
            nc.vector.memset(onesf[:], 1.0)
            ones64b = cpool.tile([1, DH], bf16, name="ones64b")
            nc.vector.memset(ones64b[:], 1.0)
            # tri01[p, j] = 1.0 where j >= p (causal-allowed), else 0
            trif = cpool.tile([128, 128], f32, name="trif")
            nc.vector.memset(trif[:], 1.0)
            nc.gpsimd.affine_select(
                out=trif[:], in_=trif[:], compare_op=ALU.is_ge, fill=0.0,
                base=0, pattern=[[1, 128]], channel_multiplier=-1,
            )
            tri01 = cpool.tile([128, 128], bf16, name="tri01")
            nc.vector.tensor_copy(tri01[:], trif[:])

            # persistent qT/kT tiles: [pair A/B][tq] each [128, 1024]
            # pair A rows 0:64 = head0 dh, 64:128 = head1; pair B = heads 2,3
            qt = [
                [qkvpool.tile([128, TQ], bf16, name=f"qt{ab}_{t}") for t in range(NTQ)]
                for ab in range(2)
            ]
            kt = [
                [qkvpool.tile([128, TQ], bf16, name=f"kt{ab}_{t}") for t in range(NTQ)]
                for ab in range(2)
            ]
            vt = [
                qkvpool.tile([128, 8 * VW], bf16, name=f"vt_{t}") for t in range(NTQ)
            ]

            def p1(tq):
                xts = []
                for ec in range(4):
                    xtile = xtpool.tile([128, TQ], bf16, name="xtile", tag="xtile")
                    nc.sync.dma_start(
                        xtile[:],
                        xT[ec * 128 : (ec + 1) * 128, tq * TQ : (tq + 1) * TQ],
                    )
                    xts.append(xtile)
                for gi, fc in enumerate((0, 2, 1, 3)):
                    dest = (qt if fc < 2 else kt)[fc % 2][tq]
                    for th in range(2):
                        tag = ("st_e", "st_o")[(gi * 2 + th) % 2]
                        ps = stpool.tile([128, 512], f32, name="mmps", tag=tag)
                        for ec in range(4):
                            nc.tensor.matmul(
                                ps[:],
                                lhsT=wqk_sb[:, ec * 512 + fc * 128 : ec * 512 + (fc + 1) * 128],
                                rhs=xts[ec][:, th * 512 : (th + 1) * 512],
                                start=(ec == 0),
                                stop=(ec == 3),
                            )
                        nc.vector.tensor_scalar_add(
                            dest[:, th * 512 : (th + 1) * 512],
                            ps[:],
                            bqk_sb[:, fc : fc + 1],
                        )
                v_tile = vt[tq]
                nc.vector.tensor_copy(
                    v_tile.rearrange("p (t h d) -> p t h d", t=8, h=HPG)[:, :, :, 64:65],
                    onesf[:, 0:32].rearrange("p (t h d) -> p t h d", t=8, h=HPG),
                )
                for tb in range(8):
                    vps = ovpool.tile(
                        [128, GQ], f32, name="vps", tag=("ov_e", "ov_o")[tb % 2]
                    )
                    for ec in range(4):
                        nc.tensor.matmul(
                            vps[:],
                            lhsT=xts[ec][:, tb * 128 : (tb + 1) * 128],
                            rhs=wv_sb[:, ec * GQ : (ec + 1) * GQ],
                            start=(ec == 0),
                            stop=(ec == 3),
                        )
                    nc.vector.tensor_tensor(
                        v_tile[:, tb * VW : (tb + 1) * VW].rearrange(
                            "p (h d) -> p h d", h=HPG
                        )[:, :, 0:64],
                        vps.rearrange("p (h d) -> p h d", h=HPG),
                        bv_bc.rearrange("p (h d) -> p h d", h=HPG),
                        ALU.add,
                    )

            atts = {}

            def epilogue(att_tile, ovt):
                # recip of the sums row -> PE-broadcast of the reciprocal into
                # the ov tile's free partitions 64:128 -> one DVE mult frees ov.
                # Every link is a RAW data dependency; no DMA on this path.
                occ = epool.tile([DH, QE], f32, name="occ", tag="occ")
                nc.vector.tensor_copy(occ[:], ovt[0:64, :])
                # custom-DVE ISA ops (reciprocal_approx_*) drop the partition
                # offset of their input AP — stage the sums row at base 0.
                srow = epool.tile([1, QE], f32, name="srow", tag="srow")
                nc.vector.tensor_copy(srow[:], ovt[64:65, :])
                rbc = epool.tile([1, QE], f32, name="rbc", tag="rbc")
                nc.vector.reciprocal_approx_fast(out=rbc[:], in_=srow[:])
                rbcb = epool.tile([1, QE], bf16, name="rbcb", tag="rbcb")
                nc.scalar.copy(rbcb[:], rbc[:])  # bf16 round on the idle ACT
                for j0, j1 in ((0, 512), (512, QE)):
                    nc.tensor.matmul(
                        ovt[64:128, j0:j1],
                        lhsT=ones64b[:],
                        rhs=rbcb[:, j0:j1],
                        start=True,
                        stop=True,
                        skip_group_check=True,
                    )
                nc.vector.tensor_tensor(
                    att_tile[:], occ[:], ovt[64:128, :], ALU.mult
                )

            def att(qq, mid=None):
                atts[qq] = [
                    attpool.tile([DH, QE], bf16, name=f"att_h{h}", tag=f"att{h}")
                    for h in range(HPG)
                ]
                nkb = 8 * qq + 8
                for pr in range(2):  # head pair (2pr, 2pr+1)
                    if pr == 1 and mid is not None:
                        mid()
                    ov_e = ovpool.tile([128, QE], f32, name="ov_e", tag="ov_e")
                    ov_o = ovpool.tile([128, QE], f32, name="ov_o", tag="ov_o")
                    for kb in range(nkb):
                        tqk, kbl = kb // 8, kb % 8
                        qs = max(0, (kb - 8 * qq) * 128)
                        st_e = stpool.tile([128, QE], f32, name="st_e", tag="st_e")
                        st_o = stpool.tile([128, QE], f32, name="st_o", tag="st_o")
                        for j0, j1 in _chunks(qs, QE):
                            # two row-tiled matmuls (rows 0:64 / 64:128)
                            nc.tensor.matmul(
                                st_e[:, j0:j1],
                                lhsT=kt[pr][tqk][0:64, kbl * 128 : (kbl + 1) * 128],
                                rhs=qt[pr][qq][0:64, j0:j1],
                                start=True,
                                stop=True,
                            )
                            nc.tensor.matmul(
                                st_o[:, j0:j1],
                                lhsT=kt[pr][tqk][64:128, kbl * 128 : (kbl + 1) * 128],
                                rhs=qt[pr][qq][64:128, j0:j1],
                                start=True,
                                stop=True,
                            )
                        pt_e = ptpool.tile([128, QE], bf16, name="pt_e", tag="pt")
                        pt_o = ptpool.tile([128, QE], bf16, name="pt_o", tag="pt")
                        nc.scalar.activation(
                            pt_e[:, qs:QE], st_e[:, qs:QE], AF.Exp, bias=0.0, scale=SCALE
                        )
                        nc.scalar.activation(
                            pt_o[:, qs:QE], st_o[:, qs:QE], AF.Exp, bias=0.0, scale=SCALE
                        )
                        if kb >= 8 * qq:  # diagonal: zero the causally-forbidden
                            for ptx in (pt_e, pt_o):  # upper triangle on DVE
                                nc.vector.tensor_tensor(
                                    ptx[:, qs : qs + 128],
                                    ptx[:, qs : qs + 128],
                                    tri01[:],
                                    ALU.mult,
                                )
                        for j0, j1 in _chunks(qs, QE):
                            nc.tensor.matmul(
                                ov_e[0:65, j0:j1],
                                lhsT=vt[tqk][:, kbl * VW + 2 * pr * 65 : kbl * VW + (2 * pr + 1) * 65],
                                rhs=pt_e[:, j0:j1],
                                start=(kb == 0),
                                stop=(kb == nkb - 1),
                                skip_group_check=True,
                            )
                            nc.tensor.matmul(
                                ov_o[0:65, j0:j1],
                                lhsT=vt[tqk][:, kbl * VW + (2 * pr + 1) * 65 : kbl * VW + (2 * pr + 2) * 65],
                                rhs=pt_o[:, j0:j1],
                                start=(kb == 0),
                                stop=(kb == nkb - 1),
                                skip_group_check=True,
                            )
                    epilogue(atts[qq][2 * pr], ov_e)
                    epilogue(atts[qq][2 * pr + 1], ov_o)

            def wo(qq):
                att_h = atts[qq]
                out_sb = opool.tile([128, 4 * 512], f32, name="out_sb", tag="osb")
                for half in range(2):
                    for tb4 in range(4):
                        tb = half * 4 + tb4
                        wops = stpool.tile(
                            [128, 512], f32, name="wops", tag=("st_e", "st_o")[tb4 % 2]
                        )
                        for h in range(HPG):
                            nc.tensor.matmul(
                                wops[:],
                                lhsT=att_h[h][:, tb * 128 : (tb + 1) * 128],
                                rhs=wo_sb[:, h * 512 : (h + 1) * 512],
                                start=(h == 0),
                                stop=(h == HPG - 1),
                            )
                        # evacuate PSUM on the scalar engine (ACT is idle here;
                        # keeps the in-order DVE queue free for epilogues)
                        nc.scalar.copy(
                            out_sb[:, tb4 * 512 : (tb4 + 1) * 512], wops[:]
                        )
                    nc.sync.dma_start(
                        out[
                            qq * QE + half * 512 : qq * QE + (half + 1) * 512, :
                        ].rearrange("(t p) c -> p t c", p=128),
                        out_sb.rearrange("p (t c) -> p t c", t=4),
                    )

            def body(_i=None):
                for tq in range(NTQ):
                    p1(tq)
                att(0)
                att(1, mid=lambda: wo(0))
                att(2, mid=lambda: wo(1))
                att(3, mid=lambda: wo(2))
                wo(3)

            if repeat == 1:
                body()
            else:
                with tc.For_i(0, repeat, 1) as _i:
                    body(_i)

    nc.finalize()
    return nc


def _get_nc(repeat=1):
    key = ("nc", repeat)
    if key not in _CACHE:
        _CACHE[key] = _build_nc(repeat)
    return _CACHE[key]


def _make_in_maps(x, Wqkv, bqkv, Wo):
    from ml_dtypes import bfloat16

    in_maps = []
    for core in range(8):
        b, g = core // 2, core % 2
        qs, ks, vs = g * GQ, 512 + g * GQ, 1024 + g * GQ
        wqk_np = np.ascontiguousarray(
            np.concatenate([Wqkv[:, qs : qs + GQ], Wqkv[:, ks : ks + GQ]], axis=1)
        ).astype(bfloat16)
        bqk_np = np.ascontiguousarray(
            np.concatenate([bqkv[qs : qs + GQ], bqkv[ks : ks + GQ]]).reshape(4, 128).T
        )
        wv_np = np.ascontiguousarray(Wqkv[:, vs : vs + GQ]).astype(bfloat16)
        bv_np = np.ascontiguousarray(bqkv[vs : vs + GQ].reshape(1, GQ)).astype(bfloat16)
        wo_g = Wo[g * GQ : (g + 1) * GQ, :]
        wo_np = np.ascontiguousarray(
            np.concatenate([wo_g[h * DH : (h + 1) * DH, :] for h in range(HPG)], axis=1)
        ).astype(bfloat16)
        in_maps.append(
            {
                "xT": np.ascontiguousarray(x[b].T).astype(bfloat16),
                "wqk": wqk_np,
                "bqk": bqk_np,
                "wv": wv_np,
                "bv": bv_np,
                "wo": wo_np,
            }
        )
    return in_maps


def kernel(x, Wqkv, bqkv, Wo, bo, **run_kwargs):
    from concourse.bass_utils import run_bass_kernel_spmd

    x = np.asarray(x, dtype=np.float32)
    Wqkv = np.asarray(Wqkv, dtype=np.float32)
    bqkv = np.asarray(bqkv, dtype=np.float32)
    Wo = np.asarray(Wo, dtype=np.float32)
    bo = np.asarray(bo, dtype=np.float32)

    nc = _get_nc()
    in_maps = _make_in_maps(x, Wqkv, bqkv, Wo)

    res = run_bass_kernel_spmd(nc, in_maps, core_ids=list(range(8)), **run_kwargs)
    _CACHE["last_results"] = res

    out = np.empty((B, S, E), dtype=np.float32)
    for b in range(B):
        out[b] = res.results[2 * b]["out"] + res.results[2 * b + 1]["out"] + bo
    return out
